# revision 24
# baseline (speedup 1.0000x reference)
"""GraphSAGE (4-layer) forward pass on 8 Trainium2 NeuronCores — v3.

Changes over v2 (baseline 3.42ms HW):
  - Single-pass S build: S = (iota == sdst) in bf16 (0/1). The 1/deg mean
    scale moves to the epilogue: agg_psum × ivdrep (a partition-replicated
    invdeg tile) on DVE, added into the self psum via an identity matmul.
    Kills the second DVE broadcast pass (~0.5 ms serial).
  - Uneven table segments [4,4,4,1] (in 1024-node group units): the last
    segment's AllGather is 2.1 MB and launches at the end of the producing
    layer, so edge processing never stalls on the big trailing chunk.
  - Trailing-pad indices are -1: the SWDGE gather ucode trims trailing
    negative idxs, skipping descriptor gen + drain for the final run's
    padding of each queue-split.
  - Half-group (4-block) PSUM tiles [128,512] (1 bank), self+agg separate;
    layer-4 pooled-agg AllReduce split in two to overlap the tail.
"""

import os
import sys
from dataclasses import dataclass

import numpy as np

for _p in ("/opt/trn_rl_repo", "/root/.axon_site/_ro/trn_rl_repo"):
    if os.path.isdir(_p) and _p not in sys.path:
        sys.path.append(_p)


def _bcast_ap(bass, t, mid, inner, expand_inner):
    """3D broadcast AP over a 2D [128, n] slice ``t``."""
    if expand_inner:
        ap = [list(t.ap[0]), [1, mid], [0, inner]]
    else:
        ap = [list(t.ap[0]), [0, mid], [1, inner]]
    return bass.AP(t.tensor, t.offset, ap)

import ml_dtypes

BF16 = ml_dtypes.bfloat16

SEGG = [4, 4, 4, 1]  # table segments, in units of (GRP*128 local nodes)
NSEG = 4


# --------------------------------------------------------------------------
# configuration
# --------------------------------------------------------------------------
@dataclass
class Cfg:
    gn: int  # nodes per graph
    gpc: list  # graphs per core (len 8)
    np_pad: int  # padded nodes per core (multiple of 1024)
    dims: list  # [d0, d1, d2, d3, d4]
    grp: int = 8  # dst blocks per group
    ncores: int = 8

    @property
    def nb(self):  # 128-node blocks per core
        return self.np_pad // 128

    @property
    def ngrp(self):
        assert self.nb % self.grp == 0
        return self.nb // self.grp

    @property
    def gb(self):  # group boundaries of segments
        b = [0]
        for s in SEGG:
            b.append(b[-1] + s)
        assert b[-1] == self.ngrp
        return b

    @property
    def segc(self):  # local rows per core per segment
        return [s * self.grp * 128 for s in SEGG]

    @property
    def segrows(self):  # table rows per segment
        r = [self.ncores * c for c in self.segc]
        assert max(r) <= 32768
        return r

    @property
    def g13(self):  # max graphs per core
        return max(self.gpc)

    @property
    def node_lo(self):
        lo = [0]
        for c in range(self.ncores):
            lo.append(lo[-1] + self.gpc[c] * self.gn)
        return lo


FULL_CFG = Cfg(
    gn=1000,
    gpc=[13, 13, 13, 13, 12, 12, 12, 12],
    np_pad=13312,
    dims=[128, 128, 118, 103, 5],
    grp=8,
)

NQ = int(os.environ.get("GATHER_QUEUES2", "4"))


# --------------------------------------------------------------------------
# host-side preprocessing
# --------------------------------------------------------------------------
def make_layout(cfg: Cfg, ncol: np.ndarray):
    """Shared column layout from the (cross-core max) ncol [ncalls, GRP].

    Per call: runs (ib) are assigned to NQ splits (greedy, descending ncol,
    to the lightest split); within a split runs are ordered ascending by
    ncol so the largest run is last (its tail padding becomes the split's
    trailing -1 region).

    Returns: per call: list of splits, each a list of ib;
             colbase [ncalls, GRP]: column offset of each run;
             split_cols [ncalls, NQ]; call_base [ncalls] (in columns).
    """
    ncalls = ncol.shape[0]
    GRP = ncol.shape[1]
    call_splits = []
    colbase = np.zeros((ncalls, GRP), np.int64)
    split_cols = np.zeros((ncalls, NQ), np.int64)
    call_cols = ncol.sum(axis=1)
    call_base = np.concatenate([[0], np.cumsum(call_cols)])
    for cid in range(ncalls):
        order = sorted(range(GRP), key=lambda ib: -ncol[cid, ib])
        splits = [[] for _ in range(NQ)]
        loads = [0] * NQ
        for ib in order:
            j = loads.index(min(loads))
            splits[j].append(ib)
            loads[j] += int(ncol[cid, ib])
        for j in range(NQ):
            splits[j].sort(key=lambda ib: ncol[cid, ib])
        c = int(call_base[cid])
        for j in range(NQ):
            split_cols[cid, j] = sum(int(ncol[cid, ib]) for ib in splits[j])
            for ib in splits[j]:
                colbase[cid, ib] = c
                c += int(ncol[cid, ib])
        call_splits.append(splits)
    return call_splits, colbase, split_cols, call_base


def preprocess(cfg: Cfg, src: np.ndarray, dst: np.ndarray):
    n = cfg.node_lo[-1]
    NB, GRP, NGRP = cfg.nb, cfg.grp, cfg.ngrp
    GB, SEGC = cfg.gb, cfg.segc
    src = np.asarray(src).astype(np.int64)
    dst = np.asarray(dst).astype(np.int64)
    deg = np.bincount(dst, minlength=n).astype(np.float64)
    invdeg = 1.0 / np.clip(deg, 1.0, None)

    lo = np.asarray(cfg.node_lo[:-1])
    core_of = np.searchsorted(np.asarray(cfg.node_lo[1:]), np.arange(n), side="right")
    local = np.arange(n) - lo[core_of]
    lg = local // (GRP * 128)  # local group id
    seg_of = np.searchsorted(np.asarray(GB[1:]), lg, side="right")
    seg_base_local = np.asarray([GB[s] * GRP * 128 for s in range(NSEG)])
    loff = local - seg_base_local[seg_of]
    segc_arr = np.asarray(SEGC)
    lidx_of = core_of * segc_arr[seg_of] + loff  # row within segment
    assert lidx_of.max() <= 32767

    # layer-4 collapse weights
    ngraphs = sum(cfg.gpc)
    gid_of = np.arange(n) // cfg.gn
    wflat = np.zeros(n * ngraphs, np.float64)
    np.add.at(wflat, src * ngraphs + gid_of[dst], invdeg[dst])
    w_all = wflat.reshape(n, ngraphs)

    ncalls = NGRP * NSEG
    out = dict(cores=[], w=[], ngraphs=ngraphs, invdeg=invdeg)
    for c in range(cfg.ncores):
        wc = np.zeros((cfg.np_pad, ngraphs), np.float64)
        wc[: cfg.node_lo[c + 1] - cfg.node_lo[c]] = w_all[
            cfg.node_lo[c] : cfg.node_lo[c + 1]
        ]
        out["w"].append(
            np.ascontiguousarray(
                wc.reshape(NB, 128, ngraphs).transpose(1, 0, 2)
            ).astype(BF16)
        )
    for c in range(cfg.ncores):
        m = (dst >= cfg.node_lo[c]) & (dst < cfg.node_lo[c + 1])
        es, ed = src[m], dst[m]
        ld = ed - cfg.node_lo[c]
        b = ld // 128
        gi = b // GRP
        ib = b - gi * GRP
        s = seg_of[es]
        lidx = lidx_of[es]
        cid = gi * NSEG + s
        order = np.lexsort((ld, ib, cid))
        es, ed, ld, b, gi, ib, s, lidx, cid = (
            x[order] for x in (es, ed, ld, b, gi, ib, s, lidx, cid)
        )
        rkey = cid * GRP + ib
        cnt = np.bincount(rkey, minlength=ncalls * GRP).reshape(ncalls, GRP)
        ncol = (cnt + 127) // 128
        if len(rkey):
            kchange = np.r_[True, rkey[1:] != rkey[:-1]]
            run_start = np.maximum.accumulate(
                np.where(kchange, np.arange(len(rkey)), 0)
            )
            rank = np.arange(len(rkey)) - run_start
        else:
            rank = np.zeros(0, np.int64)
        out["cores"].append(
            dict(cnt=cnt, ncol=ncol, rkey=rkey, rank=rank, lidx=lidx,
                 ld=ld, b=b, ed=ed)
        )
    return out


def finish_layout(cfg: Cfg, prep):
    """Build the shared layout + per-core padded arrays."""
    GRP, NGRP = cfg.grp, cfg.ngrp
    ncalls = NGRP * NSEG
    ncol = np.maximum.reduce([pc["ncol"] for pc in prep["cores"]])
    call_splits, colbase, split_cols, call_base = make_layout(cfg, ncol)
    C = int(ncol.sum())
    sl = C * 128
    invdeg = prep["invdeg"]

    # last run of each split (for -1 trailing pad)
    last_run = {}
    for cid in range(ncalls):
        for j, split in enumerate(call_splits[cid]):
            if split:
                last_run[(cid, j)] = split[-1]

    per_core = []
    for c in range(cfg.ncores):
        pc = prep["cores"][c]
        idxs = np.zeros(sl, np.int64)  # interior pad: row 0
        sdst = np.full((128, C), -1.0, np.float32)
        rb = colbase.reshape(-1)[pc["rkey"]]
        pos = rb * 128 + pc["rank"]
        idxs[pos] = pc["lidx"]
        sdst[pos % 128, pos // 128] = (pc["ld"] - pc["b"] * 128).astype(np.float32)
        # trailing -1 pad: for each (cid, split), the final run's tail
        if os.environ.get("NEG1_TRIM", "1") == "1":
            for (cid, j), ib in last_run.items():
                cbase = int(colbase[cid, ib])
                width = int(ncol[cid, ib])
                e0 = cbase * 128 + int(pc["cnt"][cid, ib])
                e1 = (cbase + width) * 128
                idxs[e0:e1] = -1
        idx16 = np.zeros((16, sl // 16), np.int16)
        p = np.arange(sl)
        idx16[p % 16, p // 16] = idxs.astype(np.int16)
        idx128 = np.tile(idx16, (8, 1))
        lo, hi = cfg.node_lo[c], cfg.node_lo[c + 1]
        ivrow = np.zeros(cfg.np_pad, np.float32)
        ivrow[: hi - lo] = invdeg[lo:hi]
        ivdrep = np.broadcast_to(
            ivrow.astype(BF16)[None, :], (128, cfg.np_pad)
        ).copy()
        per_core.append(dict(idx=idx128, sdst=sdst.astype(BF16),
                             ivdrep=ivdrep))
    return dict(ncol=ncol, call_splits=call_splits, colbase=colbase,
                split_cols=split_cols, call_base=call_base, C=C, sl=sl,
                per_core=per_core)


def pack_weights(cfg: Cfg, inp: dict):
    d = cfg.dims
    w = {}
    for l in range(1, 4):
        din, dout = d[l - 1], d[l]
        wn = np.zeros((128, 128), np.float32)
        ws = np.zeros((128, 128), np.float32)
        wn[:din, :dout] = np.asarray(inp[f"wn{l}"], np.float32)
        ws[:din, :dout] = np.asarray(inp[f"ws{l}"], np.float32)
        bb = np.zeros((128, 1), np.float32)
        bb[:dout, 0] = np.asarray(inp[f"b{l}"], np.float32)
        w[f"wn{l}"] = wn.astype(BF16)
        w[f"ws{l}"] = ws.astype(BF16)
        w[f"b{l}"] = bb
    din, dout = d[3], d[4]
    wn4 = np.zeros((128, 8), np.float32)
    ws4 = np.zeros((128, 8), np.float32)
    wn4[:din, :dout] = np.asarray(inp["wn4"], np.float32)
    ws4[:din, :dout] = np.asarray(inp["ws4"], np.float32)
    b4r = np.zeros((1, 8), np.float32)
    b4r[0, :dout] = np.asarray(inp["b4"], np.float32) * float(cfg.gn)
    w["wn4"] = wn4
    w["ws4"] = ws4
    w["b4r"] = b4r
    return w


def shard_infeat(cfg: Cfg, in_feat: np.ndarray):
    d0 = cfg.dims[0]
    shards = []
    for c in range(cfg.ncores):
        lo, hi = cfg.node_lo[c], cfg.node_lo[c + 1]
        h = np.zeros((128, cfg.np_pad), np.float32)
        h[:d0, : hi - lo] = np.asarray(in_feat[lo:hi], np.float32).T
        shards.append(h.astype(BF16))
    return shards


# --------------------------------------------------------------------------
# device program
# --------------------------------------------------------------------------
def build_nc(cfg: Cfg, layout: dict, ngraphs: int = 100,
             no_collective: bool = False):
    from concourse import bacc, bass, tile, mybir

    dt = mybir.dt
    d = cfg.dims
    NB, GRP, NGRP = cfg.nb, cfg.grp, cfg.ngrp
    NP, GN, G13 = cfg.np_pad, cfg.gn, cfg.g13
    GB, SEGC, SEGROWS = cfg.gb, cfg.segc, cfg.segrows

    ncol = layout["ncol"]
    call_splits = layout["call_splits"]
    colbase = layout["colbase"]
    split_cols = layout["split_cols"]
    call_base = layout["call_base"]
    C, sl = layout["C"], layout["sl"]
    ncalls = NGRP * NSEG
    MAXC = int(ncol.sum(axis=1).max())

    nc = bacc.Bacc(
        "TRN2",
        target_bir_lowering=False,
        debug=False,
        num_devices=cfg.ncores,
        num_swdge_queues=NQ,
        dynamic_dma_scratch_size=int(os.environ.get("DMA_SCRATCH2", "16384")),
    )

    # ---- I/O -------------------------------------------------------------
    h0t_d = nc.dram_tensor("h0t", [128, NP], dt.bfloat16, kind="ExternalInput")
    idx_d = nc.dram_tensor("idx", [128, sl // 16], dt.int16, kind="ExternalInput")
    sdst_d = nc.dram_tensor("sdst", [128, C], dt.bfloat16, kind="ExternalInput")
    ivdrep_d = nc.dram_tensor("ivdrep", [128, NP], dt.bfloat16, kind="ExternalInput")
    wpool_d = nc.dram_tensor(
        "wpool", [128, NB * ngraphs], dt.bfloat16, kind="ExternalInput"
    )
    sel_d = nc.dram_tensor("sel", [ngraphs, 16], dt.float32, kind="ExternalInput")
    wd = {}
    for l in range(1, 4):
        wd[f"wn{l}"] = nc.dram_tensor(f"wn{l}", [128, 128], dt.bfloat16, kind="ExternalInput")
        wd[f"ws{l}"] = nc.dram_tensor(f"ws{l}", [128, 128], dt.bfloat16, kind="ExternalInput")
        wd[f"b{l}"] = nc.dram_tensor(f"b{l}", [128, 1], dt.float32, kind="ExternalInput")
    wd["wn4"] = nc.dram_tensor("wn4", [128, 8], dt.float32, kind="ExternalInput")
    wd["ws4"] = nc.dram_tensor("ws4", [128, 8], dt.float32, kind="ExternalInput")
    wd["b4r"] = nc.dram_tensor("b4r", [1, 8], dt.float32, kind="ExternalInput")
    out_d = nc.dram_tensor("out", [G13, 8], dt.float32, kind="ExternalOutput")
    DBG_HT = os.environ.get("DBG_HT", "0") == "1"
    htdbg_d = None
    if DBG_HT:
        htdbg_d = [
            nc.dram_tensor(f"htdbg{l}", [128, NP], dt.bfloat16,
                           kind="ExternalOutput")
            for l in range(1, 4)
        ]

    with tile.TileContext(nc) as tc:
        with (
            tc.tile_pool(name="resident", bufs=1) as rp,
            tc.tile_pool(name="dram", bufs=1, space="DRAM") as dp,
            tc.tile_pool(name="gather", bufs=3) as gp,
            tc.tile_pool(name="spool", bufs=2) as sp,
            tc.tile_pool(name="stage", bufs=4) as stp,
            tc.tile_pool(name="tmppool", bufs=3) as tp,
            tc.tile_pool(name="psum_self", bufs=3, space="PSUM") as psl,
            tc.tile_pool(name="psum_agg", bufs=2, space="PSUM") as pag,
            tc.tile_pool(name="psum_misc", bufs=2, space="PSUM") as pms,
            tc.tile_pool(name="hts", bufs=2) as hp,
        ):
            # ---- resident SBUF tensors ----------------------------------
            idx_s = rp.tile([128, sl // 16], dt.int16)
            sdst_s = rp.tile([128, C], dt.bfloat16)
            sel_s = rp.tile([ngraphs, 16], dt.float32)
            ivdrep = rp.tile([128, NP], dt.bfloat16)
            nc.sync.dma_start(out=sel_s[:], in_=sel_d[:])
            nc.sync.dma_start(out=ivdrep[:], in_=ivdrep_d[:])
            ws_s = {}
            for k, dd in wd.items():
                if k in ("wn4", "ws4"):
                    t = rp.tile([128, 8], dt.float32, name=f"w_{k}")
                elif k == "b4r":
                    t = rp.tile([1, 8], dt.float32, name=f"w_{k}")
                elif k.startswith("b"):
                    t = rp.tile([128, 1], dt.float32, name=f"w_{k}")
                else:
                    t = rp.tile([128, 128], dt.bfloat16, name=f"w_{k}")
                ws_s[k] = t
                nc.sync.dma_start(out=t[:], in_=dd[:])
            nc.sync.dma_start(out=idx_s[:], in_=idx_d[:])
            nc.sync.dma_start(out=sdst_s[:], in_=sdst_d[:])

            # constants
            iota_i = rp.tile([128, 128], dt.int32)
            nc.gpsimd.iota(iota_i[:], pattern=[[1, 128]], base=0, channel_multiplier=0)
            iota_b = rp.tile([128, 128], dt.bfloat16)
            nc.vector.tensor_copy(iota_b[:], iota_i[:])
            pidx_i = rp.tile([128, 1], dt.int32)
            nc.gpsimd.iota(pidx_i[:], pattern=[[1, 1]], base=0, channel_multiplier=1)
            pidx_f = rp.tile([128, 1], dt.float32)
            nc.vector.tensor_copy(pidx_f[:], pidx_i[:])
            ident_b = rp.tile([128, 128], dt.bfloat16)
            nc.vector.tensor_scalar(
                ident_b[:], iota_b[:], pidx_f[:], None, mybir.AluOpType.is_equal
            )
            ones_row = rp.tile([1, G13], dt.float32)
            nc.vector.memset(ones_row[:], 1.0)

            # h tiles (transposed feature-major, bf16)
            ht = [None] * 4
            ht[0] = hp.tile([128, NP], dt.bfloat16, tag="ht", name="ht0")
            nc.sync.dma_start(out=ht[0][:], in_=h0t_d[:])

            # DRAM tables per layer-slot (0..2), per segment
            tables = [
                [
                    dp.tile([SEGROWS[s], 128], dt.bfloat16, name=f"table{t}_{s}",
                            addr_space="Local" if no_collective else "Shared")
                    for s in range(NSEG)
                ]
                for t in range(3)
            ]
            agins = [
                [
                    dp.tile([SEGC[s], 128], dt.bfloat16, name=f"agin{t}_{s}")
                    for s in range(NSEG)
                ]
                for t in range(3)
            ]
            # layer-4 pooled-agg AllReduce buffers (split in 2)
            arin = [dp.tile([128, ngraphs], dt.float32, name=f"arin{h}")
                    for h in range(2)]
            arout = [dp.tile([128, ngraphs], dt.float32, name=f"arout{h}",
                             addr_space="Local" if no_collective else "Shared")
                     for h in range(2)]

            def project_seg(l, src_ht, slot, s):
                din = d[l - 1]
                agin = agins[slot][s]
                nblk = SEGG[s] * GRP
                b0 = GB[s] * GRP
                for j in range(nblk):
                    b = b0 + j
                    st = stp.tile([128, 128], dt.bfloat16, tag="stage")
                    pp = pms.tile([128, 128], dt.float32, tag="pmisc", bufs=1)
                    nc.tensor.matmul(
                        pp[:, :],
                        src_ht[:din, b * 128 : (b + 1) * 128],
                        ws_s[f"wn{l}"][:din, :],
                        start=True,
                        stop=True,
                    )
                    nc.scalar.copy(st[:, :], pp[:, :])
                    nc.sync.dma_start(
                        out=agin[j * 128 : (j + 1) * 128, :],
                        in_=st[:, :],
                    )
                if no_collective:
                    for r in range(cfg.ncores):
                        nc.sync.dma_start(
                            out=tables[slot][s][r * SEGC[s] : (r + 1) * SEGC[s], :],
                            in_=agin[:, :],
                        )
                else:
                    nc.gpsimd.collective_compute(
                        "AllGather",
                        mybir.AluOpType.bypass,
                        replica_groups=[list(range(cfg.ncores))],
                        ins=[agin.opt()],
                        outs=[tables[slot][s].opt()],
                    )

            # per-half-group first/last edge-matmul in EMISSION order.
            # PSUM semantics: start=True resets the whole bank, so each bank
            # gets exactly one start (first MM emitted) and one stop (last).
            def hg_span(gi):
                span = {0: [None, None], 1: [None, None]}
                for s in range(NSEG):
                    cid = gi * NSEG + s
                    for j in range(NQ):
                        for ib in call_splits[cid][j]:
                            w = int(ncol[cid, ib])
                            for t in range(w):
                                key = (s, j, ib, t)
                                sp_ = span[ib // 4]
                                if sp_[0] is None:
                                    sp_[0] = key
                                sp_[1] = key
                return span

            def edge_phase(l, slot, src_ht, on_group_done):
                din = d[l - 1]
                dout = d[l]
                tq = tables[slot]
                for gi in range(NGRP):
                    # half-group psums: [128, 512] 1 bank each
                    selfp = [
                        psl.tile([128, 512], dt.float32, tag="selfp",
                                 name=f"selfp_{l}_{gi}_{h}")
                        for h in range(2)
                    ]
                    aggp = [
                        pag.tile([128, 512], dt.float32, tag="aggp",
                                 name=f"aggp_{l}_{gi}_{h}")
                        for h in range(2)
                    ]
                    spans = hg_span(gi)
                    # self matmuls (stationary ws shared); one start/stop
                    # pair per selfp bank
                    for ib in range(GRP):
                        b = gi * GRP + ib
                        pslice = selfp[ib // 4][:, (ib % 4) * 128 : (ib % 4 + 1) * 128]
                        nc.tensor.matmul(
                            pslice[:, :],
                            ws_s[f"ws{l}"][:din, :],
                            src_ht[:din, b * 128 : (b + 1) * 128],
                            start=(ib % 4 == 0),
                            stop=(ib % 4 == 3),
                        )
                    for s in range(NSEG):
                        cid = gi * NSEG + s
                        ccols = int(ncol[cid].sum())
                        gt = gp.tile([128, MAXC, 128], dt.bfloat16, tag="gather")
                        # gathers: one per split, distinct queues
                        cc0 = 0
                        for j in range(NQ):
                            scols = int(split_cols[cid, j])
                            if scols == 0:
                                continue
                            e0 = (int(call_base[cid]) + cc0) * 128
                            nidx = scols * 128
                            nc.gpsimd.dma_gather(
                                gt[:, cc0 : cc0 + scols, :],
                                tq[s][:, :],
                                idx_s[:, e0 // 16 : (e0 + nidx) // 16],
                                nidx,
                                nidx,
                                128,
                                elem_step=128,
                                single_packet=False,
                                queue_num=j,
                            )
                            cc0 += scols
                        # S build: single pass (iota == sdst)
                        b0 = int(call_base[cid])
                        key2 = sdst_s[:, b0 : b0 + ccols]
                        sbig = sp.tile([128, MAXC, 128], dt.bfloat16, tag="S")
                        nc.vector.scalar_tensor_tensor(
                            sbig[:, :ccols, :],
                            _bcast_ap(bass, iota_b[:, :], ccols, 128, False),
                            1.0,
                            _bcast_ap(bass, key2, ccols, 128, True),
                            mybir.AluOpType.mult,
                            mybir.AluOpType.is_equal,
                        )
                        # edge matmuls, in shared-layout order
                        for j in range(NQ):
                            for ib in call_splits[cid][j]:
                                w = int(ncol[cid, ib])
                                cb = int(colbase[cid, ib]) - b0
                                first, last = spans[ib // 4]
                                ptile = aggp[ib // 4]
                                pslice = ptile[
                                    :, (ib % 4) * 128 : (ib % 4 + 1) * 128
                                ]
                                for t in range(w):
                                    col = cb + t
                                    key = (s, j, ib, t)
                                    nc.tensor.matmul(
                                        pslice[:, :],
                                        gt[:, col, :],
                                        sbig[:, col, :],
                                        start=(first == key),
                                        stop=(last == key),
                                    )
                    # epilogue per half-group: scale agg by invdeg (DVE),
                    # add the self psum (DVE), relu+bias out (ACT)
                    for h in range(2):
                        c0 = (gi * GRP + h * 4) * 128
                        tmp = tp.tile([128, 512], dt.bfloat16, tag="tmp")
                        nc.vector.scalar_tensor_tensor(
                            tmp[:, :],
                            aggp[h][:, :],
                            1.0,
                            ivdrep[:, c0 : c0 + 512],
                            mybir.AluOpType.mult,
                            mybir.AluOpType.mult,
                        )
                        comb = tp.tile([128, 512], dt.bfloat16, tag="comb")
                        nc.vector.scalar_tensor_tensor(
                            comb[:, :],
                            selfp[h][:, :],
                            1.0,
                            tmp[:, :],
                            mybir.AluOpType.mult,
                            mybir.AluOpType.add,
                        )
                        nc.scalar.activation(
                            ht[l][:dout, c0 : c0 + 512],
                            comb[:dout, :],
                            mybir.ActivationFunctionType.Relu,
                            bias=ws_s[f"b{l}"][:dout, 0:1],
                        )
                    on_group_done(gi)

            # ---------------- main schedule ------------------------------
            def make_cb(l_next, slot_next):
                ready = {GB[s + 1] - 1: s for s in range(NSEG)}

                def cb(gi):
                    if gi in ready:
                        project_seg(l_next, ht[l_next - 1], slot_next, ready[gi])

                return cb

            def _sched():
                d3 = d[3]
                for s in range(NSEG):
                    project_seg(1, ht[0], 0, s)
                ht[1] = hp.tile([128, NP], dt.bfloat16, tag="ht", name="ht1")
                edge_phase(1, 0, ht[0], make_cb(2, 1))
                if DBG_HT:
                    nc.sync.dma_start(out=htdbg_d[0][:], in_=ht[1][:])

                ht[2] = hp.tile([128, NP], dt.bfloat16, tag="ht", name="ht2")
                edge_phase(2, 1, ht[1], make_cb(3, 2))
                if DBG_HT:
                    nc.sync.dma_start(out=htdbg_d[1][:], in_=ht[2][:])

                # layer-4 pooled aggregation, split into two AllReduces
                ppagg = [
                    pms.tile([128, ngraphs], dt.float32, tag="l4agg",
                             name=f"l4agg{h}", bufs=1)
                    for h in range(2)
                ]
                GSPLIT = 6  # groups 0..5 -> half 0; 6..12 -> half 1
                BSPLIT = GSPLIT * GRP
                arbuf = [rp.tile([128, ngraphs], dt.float32, name=f"arbuf{h}")
                         for h in range(2)]

                def l4_flush(h):
                    nc.scalar.copy(arbuf[h][:, :], ppagg[h][:, :])
                    nc.sync.dma_start(out=arin[h][:, :], in_=arbuf[h][:, :])
                    if no_collective:
                        nc.sync.dma_start(out=arout[h][:, :], in_=arin[h][:, :])
                    else:
                        nc.gpsimd.collective_compute(
                            "AllReduce",
                            mybir.AluOpType.add,
                            replica_groups=[list(range(cfg.ncores))],
                            ins=[arin[h].opt()],
                            outs=[arout[h].opt()],
                        )

                def l4cb(gi):
                    h = 0 if gi < GSPLIT else 1
                    blo = 0 if h == 0 else BSPLIT
                    bhi = BSPLIT if h == 0 else NB
                    wp = stp.tile([128, GRP * ngraphs], dt.bfloat16, tag="wp",
                                  bufs=2)
                    nc.sync.dma_start(
                        out=wp[:, :],
                        in_=wpool_d[:, gi * GRP * ngraphs : (gi + 1) * GRP * ngraphs],
                    )
                    for ib in range(GRP):
                        b = gi * GRP + ib
                        ppb = pms.tile([128, 128], dt.bfloat16, tag="projT",
                                       bufs=1)
                        nc.tensor.transpose(
                            ppb[:, :d3],
                            ht[3][:d3, b * 128 : (b + 1) * 128],
                            ident_b[:d3, :d3],
                        )
                        st = stp.tile([128, 128], dt.bfloat16, tag="stage")
                        nc.scalar.copy(st[:, :d3], ppb[:, :d3])
                        nc.tensor.matmul(
                            ppagg[h][:, :],
                            st[:, :],
                            wp[:, ib * ngraphs : (ib + 1) * ngraphs],
                            start=(b == blo),
                            stop=(b == bhi - 1),
                        )
                    if gi == GSPLIT - 1:
                        l4_flush(0)
                    if gi == NGRP - 1:
                        l4_flush(1)

                ht[3] = hp.tile([128, NP], dt.bfloat16, tag="ht", name="ht3")
                edge_phase(3, 2, ht[2], l4cb)
                if DBG_HT:
                    nc.sync.dma_start(out=htdbg_d[2][:], in_=ht[3][:])

                # combine the two AllReduce halves
                par0 = rp.tile([128, ngraphs], dt.float32)
                par1 = rp.tile([128, ngraphs], dt.float32)
                nc.sync.dma_start(out=par0[:, :], in_=arout[0][:, :])
                nc.sync.dma_start(out=par1[:, :], in_=arout[1][:, :])
                pagg_s = rp.tile([128, ngraphs], dt.float32)
                nc.vector.scalar_tensor_tensor(
                    pagg_s[:, :], par0[:, :], 1.0, par1[:, :],
                    mybir.AluOpType.mult, mybir.AluOpType.add,
                )

                pf100 = pms.tile([ngraphs, 8], dt.float32, tag="pmisc", bufs=1)
                nc.tensor.matmul(
                    pf100[:, : d[4]],
                    pagg_s[:d3, :ngraphs],
                    ws_s["wn4"][:d3, : d[4]],
                    start=True,
                    stop=True,
                )
                pf100_s = rp.tile([ngraphs, 8], dt.float32)
                nc.scalar.copy(pf100_s[:, :], pf100[:, :])

                ph3 = rp.tile([128, G13], dt.float32)
                for g in range(G13):
                    nc.vector.tensor_reduce(
                        ph3[:d3, g : g + 1],
                        ht[3][:d3, g * GN : (g + 1) * GN],
                        mybir.AxisListType.X,
                        mybir.AluOpType.add,
                    )

                pf = pms.tile([G13, 8], dt.float32, tag="pmisc", bufs=1)
                nc.tensor.matmul(
                    pf[:, : d[4]], ph3[:d3, :G13], ws_s["ws4"][:d3, : d[4]],
                    start=True, stop=False,
                )
                nc.tensor.matmul(
                    pf[:, : d[4]],
                    sel_s[:ngraphs, :G13],
                    pf100_s[:ngraphs, : d[4]],
                    start=False, stop=False,
                )
                nc.tensor.matmul(
                    pf[:, : d[4]], ones_row[0:1, :G13], ws_s["b4r"][0:1, : d[4]],
                    start=False, stop=True,
                )
                outs = rp.tile([G13, 8], dt.float32)
                nc.vector.tensor_scalar(
                    outs[:, : d[4]], pf[:, : d[4]], 1.0 / GN, None,
                    mybir.AluOpType.mult,
                )
                nc.sync.dma_start(out=out_d[:, : d[4]], in_=outs[:, : d[4]])

            _sched()

    nc.compile()
    return nc


# --------------------------------------------------------------------------
# driver
# --------------------------------------------------------------------------
def make_in_maps(cfg: Cfg, inputs: dict):
    prep = preprocess(cfg, inputs["src"], inputs["dst"])
    layout = finish_layout(cfg, prep)
    w = pack_weights(cfg, inputs)
    shards = shard_infeat(cfg, inputs["in_feat"])
    in_maps = []
    ngraphs = prep["ngraphs"]
    for c in range(cfg.ncores):
        pcl = layout["per_core"][c]
        g0 = sum(cfg.gpc[:c])
        sel = np.zeros((ngraphs, 16), np.float32)
        for j in range(cfg.gpc[c]):
            sel[g0 + j, j] = 1.0
        m = dict(
            h0t=shards[c],
            idx=pcl["idx"],
            sdst=pcl["sdst"],
            ivdrep=pcl["ivdrep"],
            wpool=prep["w"][c].reshape(128, -1),
            sel=sel,
        )
        m.update(w)
        in_maps.append(m)
    prepinfo = dict(
        ncol=layout["ncol"], C=layout["C"], sl=layout["sl"],
        ngraphs=ngraphs, layout=layout,
    )
    return prepinfo, in_maps


def assemble_output(cfg: Cfg, results):
    ngraphs = sum(cfg.gpc)
    out = np.zeros((ngraphs, cfg.dims[4]), np.float32)
    g0 = 0
    for c in range(cfg.ncores):
        r = results[c]["out"]
        out[g0 : g0 + cfg.gpc[c]] = np.asarray(r, np.float32)[: cfg.gpc[c], : cfg.dims[4]]
        g0 += cfg.gpc[c]
    return out


_CACHE = {}


def kernel(**inputs) -> np.ndarray:
    cfg = FULL_CFG
    prep, in_maps = make_in_maps(cfg, inputs)
    key = ("nc3", prep["ncol"].tobytes())
    if key not in _CACHE:
        _CACHE[key] = build_nc(cfg, prep["layout"], prep["ngraphs"])
    nc = _CACHE[key]
    from concourse.bass_utils import run_bass_kernel_spmd

    res = run_bass_kernel_spmd(nc, in_maps, core_ids=list(range(cfg.ncores)))
    return assemble_output(cfg, res.results)
